# revision 1
# baseline (speedup 1.0000x reference)
"""Trainium2 Bass kernel for nn_Attention_88441966559243.

Attention with additive bias [B,N,N] and per-key bool mask, fp32.
  B=2, N=2048, QD=1024, HEADS=16, DIM_HEAD=64.

Sharding: 8 cores = (batch b = core//4) x (query slice q0 = (core%4)*512).
Each core computes out[b, q0:q0+512, :] completely on-device; the host gather
is a pure concatenation. No collectives.

Per-core pipeline (all matmuls in float32r = full-rate ~tf32):
  A. transpose x on-chip via PE -> xT [1024,2048]; project qT (pre-scaled by
     1/8, SBUF-resident), kT -> DRAM scratch, and v' = [v | 1] (per-head ones
     column gives the softmax denominator for free) -> DRAM scratch.
  B. transpose the bias slice -> biasT [2048 keys, 512 queries], SBUF-resident.
  C. per head-pair (kT/v' slabs streamed back with full-partition, >=512B-row
     DMAs): sim^T chunks accumulated in PSUM on top of a PE-injected bias copy
     (identity matmul seeds has_written), masked exp in one ACT pass with the
     key mask as a per-partition bias vector, then U^T = v'^T @ e^T with the
     e@v matmuls lagged 6 chunks behind the exp producers (SBUF-buffered e^T)
     so the in-order PE queue never blocks on ACT.  Normalization via
     reciprocal_approx_fast of the PE-replicated denominator row.
  D. out = out_merged @ Wo + bo with per-head K=64 accumulation chunks.

Measured on HW (8 cores, For_i-loop timing): ~650 us/invocation,
rel err vs fp32 jax reference 4.4e-4 (float32r rounding).
"""
import sys
for _p in ("/opt/trn_rl_repo", "/root/.axon_site/_ro/trn_rl_repo"):
    if _p not in sys.path:
        sys.path.insert(0, _p)

import numpy as np

import concourse.bass as bass
import concourse.mybir as mybir
from concourse import bacc
from concourse.tile import TileContext
from concourse.masks import make_identity
from concourse.bass_utils import run_bass_kernel_spmd

F = 1024          # feature dim (QD == INNER)
NK = 2048         # keys (full sequence)
Q = 512           # queries per core
H = 16            # heads
D = 64            # head dim
DV = 65           # head dim + ones column
SCALE = D ** -0.5
MASK_NEG = -30000.0

FC = F // 128      # 8 feature chunks
KC = NK // 128     # 16 key chunks
NB = NK // 512     # 4 key 512-blocks

f32 = mybir.dt.float32
fr = mybir.dt.float32r
AF = mybir.ActivationFunctionType


import os


def build_nc(niter: int = 1, STAGES: str = "ABCD", ABL: str = ""):
    nc = bacc.Bacc(None, target_bir_lowering=False)

    x_in = nc.dram_tensor("x_in", [NK, F], f32, kind="ExternalInput")
    xq_in = nc.dram_tensor("xq_in", [Q, F], f32, kind="ExternalInput")
    bias_in = nc.dram_tensor("bias_in", [Q, NK], f32, kind="ExternalInput")
    maskneg_in = nc.dram_tensor("maskneg_in", [128, KC], f32, kind="ExternalInput")
    wq_in = nc.dram_tensor("wq_in", [F, F], fr, kind="ExternalInput")
    wkv_in = nc.dram_tensor("wkv_in", [F, 2 * F], fr, kind="ExternalInput")
    wo_in = nc.dram_tensor("wo_in", [F, F], fr, kind="ExternalInput")
    bo_in = nc.dram_tensor("bo_in", [1, F], fr, kind="ExternalInput")
    out_t = nc.dram_tensor("out_t", [Q, F], f32, kind="ExternalOutput")

    with TileContext(nc) as tc:
        with (
            tc.tile_pool(name="const", bufs=1) as constp,
            tc.tile_pool(name="dram", bufs=1, space="DRAM") as dramp,
            tc.tile_pool(name="ps", bufs=(4 if ABL == "quad" else 6), space="PSUM") as psA,
            tc.tile_pool(name="psu", bufs=(4 if ABL == "quad" else 2), space="PSUM") as psUp,
        ):
            # ---- constants ----
            ident = constp.tile([128, 128], f32)
            make_identity(nc, ident)
            ident_r = constp.tile([128, 128], fr)
            nc.scalar.copy(ident_r[:, :], ident[:, :])
            ident_q = constp.tile([128, 128], fr)
            nc.scalar.mul(ident_q[:, :], ident[:, :], 0.25)
            ones_f = constp.tile([128, 128], f32)
            nc.vector.memset(ones_f[:, :], 1.0)
            ones_r = constp.tile([128, 128], fr)
            nc.scalar.copy(ones_r[:, :], ones_f[:, :])
            masksb = constp.tile([128, KC], f32)
            nc.sync.dma_start(masksb[:, :], maskneg_in[:, :])
            bo_sb = constp.tile([1, F], fr)
            nc.sync.dma_start(bo_sb[:, :], bo_in[:, :])
            bo_rep = constp.tile([128, F], f32)

            vprime = dramp.tile([NK, H * DV], fr)       # v' (keys-major)
            kTd = dramp.tile([F, NK], fr)               # k^T
            outM = dramp.tile([F, Q], fr)               # merged out^T

            def body(_iv=None):
                with tc.tile_pool(name="qTp", bufs=1) as qTp:
                    qT = [qTp.tile([128, Q], fr, tag=f"qT{i}", name=f"qT{i}")
                          for i in range(FC)]

                    # ======== stage A ========
                    with (
                        tc.tile_pool(name="wload", bufs=8) as wlp,
                        tc.tile_pool(name="xTp", bufs=1) as xTp,
                        tc.tile_pool(name="kst", bufs=3) as kstp,
                    ):
                        xT = [xTp.tile([128, NK], fr, tag=f"xT{i}", name=f"xT{i}")
                              for i in range(FC)]

                        # A2 weights can load immediately
                        wq = [wlp.tile([128, F], fr, tag="w", name="w")
                              for _ in range(FC)]
                        for fc in range(FC):
                            nc.sync.dma_start(wq[fc][:, :],
                                              wq_in[fc * 128:(fc + 1) * 128, :])

                        with tc.tile_pool(name="xqTp", bufs=1) as xqTp:
                            xqT = [xqTp.tile([128, Q], fr, tag=f"xqT{i}",
                                             name=f"xqT{i}") for i in range(FC)]
                            # ---- A1: transpose x -> xT, xq -> xqT ----
                            with tc.tile_pool(name="xn", bufs=5) as xnp:
                                for rg in range(4):
                                    xns = []
                                    for r4 in range(4):
                                        rc = rg * 4 + r4
                                        xn = xnp.tile([128, F], f32, name="xn")
                                        nc.sync.dma_start(
                                            xn[:, :],
                                            x_in[rc * 128:(rc + 1) * 128, :])
                                        xns.append(xn)
                                    for fc in range(FC):
                                        ps = psA.tile([128, 512], f32, name="psa")
                                        for r4 in range(4):
                                            nc.tensor.transpose(
                                                ps[:, r4 * 128:(r4 + 1) * 128],
                                                xns[r4][:, fc * 128:(fc + 1) * 128],
                                                ident[:, :])
                                        nc.scalar.copy(
                                            xT[fc][:, rg * 512:(rg + 1) * 512],
                                            ps[:, :])
                                xqs = []
                                for r4 in range(4):
                                    xn = xnp.tile([128, F], f32, name="xn")
                                    nc.sync.dma_start(
                                        xn[:, :], xq_in[r4 * 128:(r4 + 1) * 128, :])
                                    xqs.append(xn)
                                for fc in range(FC):
                                    ps = psA.tile([128, 512], f32, name="psa")
                                    for r4 in range(4):
                                        nc.tensor.transpose(
                                            ps[:, r4 * 128:(r4 + 1) * 128],
                                            xqs[r4][:, fc * 128:(fc + 1) * 128],
                                            ident[:, :])
                                    nc.scalar.copy(xqT[fc][:, :], ps[:, :])

                            # ---- A2: qT = (Wq^T @ xqT) * SCALE ----
                            for m in range(FC):
                                ps = psA.tile([128, 512], f32, name="psa")
                                for fc in range(FC):
                                    nc.tensor.matmul(
                                        ps[:, :],
                                        wq[fc][:, m * 128:(m + 1) * 128],
                                        xqT[fc][:, :],
                                        start=(fc == 0), stop=(fc == FC - 1))
                                nc.scalar.mul(qT[m][:, :], ps[:, :], SCALE)

                        # ---- A3: kT = Wk^T @ xT -> DRAM ----
                        wk = [wlp.tile([128, F], fr, tag="w", name="w")
                              for _ in range(FC)]
                        for fc in range(FC):
                            nc.sync.dma_start(
                                wk[fc][:, :], wkv_in[fc * 128:(fc + 1) * 128, 0:F])
                        for m in range(FC):
                            kst = kstp.tile([128, NK], fr, name="kst")
                            for nb in range(NB):
                                ps = psA.tile([128, 512], f32, name="psa")
                                for fc in range(FC):
                                    nc.tensor.matmul(
                                        ps[:, :],
                                        wk[fc][:, m * 128:(m + 1) * 128],
                                        xT[fc][:, nb * 512:(nb + 1) * 512],
                                        start=(fc == 0), stop=(fc == FC - 1))
                                nc.scalar.copy(kst[:, nb * 512:(nb + 1) * 512],
                                               ps[:, :])
                            nc.sync.dma_start(kTd[m * 128:(m + 1) * 128, :],
                                              kst[:, :])

                        # ---- A4: v' = [x @ Wv | 1] -> DRAM ----
                        wv = [wlp.tile([128, F], fr, tag="w", name="w")
                              for _ in range(FC)]
                        for fc in range(FC):
                            nc.sync.dma_start(
                                wv[fc][:, :],
                                wkv_in[fc * 128:(fc + 1) * 128, F:2 * F])
                        with tc.tile_pool(name="vst", bufs=3) as vstp:
                            for kc in range(KC):
                                vst = vstp.tile([128, H * DV], fr, name="vst")
                                for half in range(2):
                                    ps = psA.tile([128, 512], f32, name="psa")
                                    for fc in range(FC):
                                        nc.tensor.matmul(
                                            ps[:, :],
                                            xT[fc][:, kc * 128:(kc + 1) * 128],
                                            wv[fc][:, half * 512:(half + 1) * 512],
                                            start=(fc == 0), stop=(fc == FC - 1))
                                    dst = vst[:, half * 8 * DV:(half + 1) * 8 * DV] \
                                        .rearrange("p (h x) -> p h x", x=DV)[:, :, 0:64]
                                    nc.scalar.copy(
                                        dst,
                                        ps[:, :].rearrange("p (h d) -> p h d", d=64))
                                ones_dst = vst[:, :].rearrange(
                                    "p (h x) -> p h x", x=DV)[:, :, 64:65]
                                nc.vector.tensor_copy(
                                    ones_dst,
                                    ones_r[:, 0:H].rearrange("p (a b) -> p a b", b=1))
                                nc.sync.dma_start(
                                    vprime[kc * 128:(kc + 1) * 128, :], vst[:, :])

                    if "B" not in STAGES:
                        with tc.tile_pool(name="dbg", bufs=2) as dbgp:
                            dbg = dbgp.tile([128, 512], fr, name="dbg")
                            nc.sync.dma_start(dbg[:, :], kTd[0:128, 0:512])
                            dbf = dbgp.tile([128, 512], f32, name="dbf")
                            nc.vector.tensor_copy(dbf[:, :], dbg[:, :])
                            nc.sync.dma_start(out_t[0:128, 0:512], dbf[:, :])
                        return

                    # ======== stages B + C ========
                    with tc.tile_pool(name="biasTp", bufs=1) as biasTp:
                        biasT = [biasTp.tile([128, Q], fr, tag=f"bT{i}",
                                             name=f"bT{i}") for i in range(KC)]
                        # ---- B: bias transpose ----
                        with tc.tile_pool(name="bn", bufs=4) as bnp:
                            bns = []
                            for qc in range(4):
                                bn = bnp.tile([128, NK], f32, name="bn")
                                nc.sync.dma_start(
                                    bn[:, :], bias_in[qc * 128:(qc + 1) * 128, :])
                                bns.append(bn)
                            for kc in range(KC):
                                ps = psA.tile([128, 512], f32, name="psa")
                                for qc in range(4):
                                    nc.tensor.transpose(
                                        ps[:, qc * 128:(qc + 1) * 128],
                                        bns[qc][:, kc * 128:(kc + 1) * 128],
                                        ident[:, :])
                                nc.scalar.copy(biasT[kc][:, :], ps[:, :])

                        if "C" not in STAGES:
                            with tc.tile_pool(name="dbg", bufs=2) as dbgp:
                                dbf = dbgp.tile([128, 512], f32, name="dbf")
                                nc.vector.tensor_copy(dbf[:, :], biasT[0][:, :])
                                nc.sync.dma_start(out_t[0:128, 0:512], dbf[:, :])
                            return

                        # ---- C: attention, head pairs ----
                        with (
                            tc.tile_pool(name="vph", bufs=2) as vphp,
                            tc.tile_pool(name="kph", bufs=2) as kphp,
                            tc.tile_pool(name="et", bufs=(12 if ABL == "quad" else 10)) as ep,
                            tc.tile_pool(name="dsb", bufs=2) as dsbp,
                            tc.tile_pool(name="rrep", bufs=2) as rrepp,
                            tc.tile_pool(name="otst", bufs=2) as otstp,
                        ):
                          if ABL == "quad":
                            for hq in range(H // 4):
                                vph = vphp.tile([128, KC * 4 * DV], fr, name="vph")
                                nc.sync.dma_start(
                                    vph[:, :].rearrange("p (kc d) -> p kc d",
                                                        d=4 * DV),
                                    vprime[:, 4 * hq * DV:(4 * hq + 4) * DV]
                                    .rearrange("(kc p) d -> p kc d", p=128))
                                kphs = []
                                for pp in range(2):
                                    kph = kphp.tile([128, NK], fr, name="kph")
                                    nc.sync.dma_start(
                                        kph[:, :],
                                        kTd[(2 * hq + pp) * 128:
                                            (2 * hq + pp + 1) * 128, :])
                                    kphs.append(kph)
                                psU4 = [psUp.tile([DV, 512], f32, name="psu")
                                        for _ in range(4)]
                                pending = []

                                def drain_av(upto):
                                    while pending and pending[0][0] <= upto:
                                        kc0, eTs = pending.pop(0)
                                        for sub in range(4):
                                            nc.tensor.matmul(
                                                psU4[sub][:, :],
                                                vph[:, kc0 * 4 * DV + sub * DV:
                                                    kc0 * 4 * DV + (sub + 1) * DV],
                                                eTs[sub][:, :],
                                                start=(kc0 == 0),
                                                stop=(kc0 == KC - 1))

                                for kc in range(KC):
                                    pss, eTs = [], []
                                    for sub in range(4):
                                        po = (sub % 2) * 64
                                        ps = psA.tile([128, 512], f32, name="psa")
                                        nc.tensor.matmul(ps[:, :], ident_r[:, :],
                                                         biasT[kc][:, :],
                                                         start=True, stop=False)
                                        nc.tensor.matmul(
                                            ps[:, :],
                                            kphs[sub // 2][po:po + 64,
                                                           kc * 128:(kc + 1) * 128],
                                            qT[2 * hq + sub // 2][po:po + 64, :],
                                            start=False, stop=True)
                                        pss.append(ps)
                                    for sub in range(4):
                                        eT = ep.tile([128, 512], fr, name="eT")
                                        nc.scalar.activation(
                                            eT[:, :], pss[sub][:, :], AF.Exp,
                                            bias=masksb[:, kc:kc + 1], scale=1.0)
                                        eTs.append(eT)
                                    pending.append((kc, eTs))
                                    drain_av(kc - 2)
                                drain_av(KC)
                                for sub in range(4):
                                    h = 4 * hq + sub
                                    psU = psU4[sub]
                                    Dsb = dsbp.tile([DV, 512], fr, name="Dsb")
                                    nc.scalar.copy(Dsb[64:65, :], psU[64:65, :])
                                    psR = psA.tile([128, 512], f32, name="psa")
                                    nc.tensor.matmul(psR[0:64, :],
                                                     ones_r[64:65, 0:64],
                                                     Dsb[64:65, :],
                                                     start=True, stop=True)
                                    rrep = rrepp.tile([64, 512], f32, name="rrep")
                                    nc.vector.reciprocal_approx_fast(
                                        out=rrep[:, :], in_=psR[0:64, :])
                                    ot = otstp.tile([64, Q], fr, name="ot")
                                    nc.vector.tensor_mul(ot[:, :], psU[0:64, :],
                                                         rrep[:, :])
                                    nc.sync.dma_start(
                                        outM[h * 64:(h + 1) * 64, :], ot[:, :])
                          else:
                            KCC = KC // 2 if ABL == "halfkc" else KC
                            hoisted = [None, None]
                            for hp in range(H // 2):
                                if ABL == "hoistdma" and hoisted[0] is not None:
                                    vph, kph = hoisted
                                else:
                                    # paired loads: full partitions, >=512B rows
                                    vph = vphp.tile([128, KC * 2 * DV], fr, name="vph")
                                    nc.sync.dma_start(
                                        vph[:, :].rearrange("p (kc d) -> p kc d",
                                                            d=2 * DV),
                                        vprime[:, 2 * hp * DV:(2 * hp + 2) * DV]
                                        .rearrange("(kc p) d -> p kc d", p=128))
                                    kph = kphp.tile([128, NK], fr, name="kph")
                                    nc.sync.dma_start(
                                        kph[:, :],
                                        kTd[hp * 128:(hp + 1) * 128, :])
                                    if ABL == "hoistdma":
                                        hoisted = [vph, kph]
                                psU2 = [psUp.tile([DV, 512], f32, name="psu")
                                        for _ in range(2)]
                                # lag e@v one chunk behind sim/exp so the
                                # in-order PE queue never waits on ACT.
                                pending = []

                                def drain_av(upto):
                                    while pending and pending[0][0] <= upto:
                                        kc0, eTs = pending.pop(0)
                                        for sub in range(2):
                                            nc.tensor.matmul(
                                                psU2[sub][:, :],
                                                vph[:, kc0 * 2 * DV + sub * DV:
                                                    kc0 * 2 * DV + (sub + 1) * DV],
                                                eTs[sub][:, :],
                                                start=(kc0 == 0),
                                                stop=(kc0 == KCC - 1))

                                for kc in range(KCC):
                                    pss, eTs = [], []
                                    for sub in range(2):
                                        po = sub * 64
                                        ps = psA.tile([128, 512], f32, name="psa")
                                        # bias injected as 4 quarter-
                                        # strength identity matmuls: the extra
                                        # PE passes are redundant FLOP-wise
                                        # but keep enough PE work in flight
                                        # per PSUM slot to hide the producer->
                                        # consumer round-trip latency.
                                        for ks in range(4):
                                            nc.tensor.matmul(
                                                ps[:, :], ident_q[:, :],
                                                biasT[kc][:, :],
                                                start=(ks == 0), stop=False)
                                        nc.tensor.matmul(
                                            ps[:, :],
                                            kph[po:po + 64,
                                                kc * 128:(kc + 1) * 128],
                                            qT[hp][po:po + 64, :],
                                            start=False, stop=True)
                                        pss.append(ps)
                                    for sub in range(2):
                                        eT = ep.tile([128, 512], fr, name="eT")
                                        if ABL == "dvecopy":
                                            nc.vector.tensor_copy(eT[:, :],
                                                                  pss[sub][:, :])
                                        elif ABL == "nobias":
                                            nc.scalar.activation(
                                                eT[:, :], pss[sub][:, :], AF.Exp,
                                                scale=1.0)
                                        else:
                                            nc.scalar.activation(
                                                eT[:, :], pss[sub][:, :], AF.Exp,
                                                bias=masksb[:, kc:kc + 1], scale=1.0)
                                        eTs.append(eT)
                                    pending.append((kc, eTs))
                                    # batched drain: one PE wait covers 4 avs (ACT in-order)
                                    if kc >= 9 and (kc - 9) % 4 == 3:
                                        drain_av(kc - 6)
                                drain_av(KCC)
                                for sub in range(2):
                                    h = 2 * hp + sub
                                    psU = psU2[sub]
                                    Dsb = dsbp.tile([DV, 512], fr, name="Dsb")
                                    nc.scalar.copy(Dsb[64:65, :], psU[64:65, :])
                                    psR = psA.tile([128, 512], f32, name="psa")
                                    nc.tensor.matmul(psR[0:64, :],
                                                     ones_r[64:65, 0:64],
                                                     Dsb[64:65, :],
                                                     start=True, stop=True)
                                    rrep = rrepp.tile([64, 512], f32, name="rrep")
                                    nc.vector.reciprocal_approx_fast(
                                        out=rrep[:, :], in_=psR[0:64, :])
                                    ot = otstp.tile([64, Q], fr, name="ot")
                                    nc.vector.tensor_mul(ot[:, :], psU[0:64, :],
                                                         rrep[:, :])
                                    nc.sync.dma_start(
                                        outM[h * 64:(h + 1) * 64, :], ot[:, :])

                if "D" not in STAGES:
                    with tc.tile_pool(name="dbg", bufs=2) as dbgp:
                        dbg = dbgp.tile([128, 512], fr, name="dbg")
                        nc.sync.dma_start(dbg[:, :], outM[0:128, :])
                        dbf = dbgp.tile([128, 512], f32, name="dbf")
                        nc.vector.tensor_copy(dbf[:, :], dbg[:, :])
                        nc.sync.dma_start(out_t[0:128, 0:512], dbf[:, :])
                    return

                # ======== stage D ========
                with (
                    tc.tile_pool(name="wop", bufs=1) as wop,
                    tc.tile_pool(name="oMp", bufs=1) as oMp,
                    tc.tile_pool(name="fin", bufs=3) as finp,
                ):
                    wo = [wop.tile([64, F], fr, tag=f"wo{i}", name=f"wo{i}")
                          for i in range(H)]
                    oM = [oMp.tile([64, Q], fr, tag=f"oM{i}", name=f"oM{i}")
                          for i in range(H)]
                    for i in range(H):
                        nc.sync.dma_start(wo[i][:, :],
                                          wo_in[i * 64:(i + 1) * 64, :])
                        nc.sync.dma_start(oM[i][:, :],
                                          outM[i * 64:(i + 1) * 64, :])
                    for nb2 in range(2):
                        ps = psA.tile([128, 512], f32, name="psa")
                        nc.tensor.matmul(ps[:, :], ones_r[0:1, 0:128],
                                         bo_sb[0:1, nb2 * 512:(nb2 + 1) * 512],
                                         start=True, stop=True)
                        nc.scalar.copy(bo_rep[:, nb2 * 512:(nb2 + 1) * 512],
                                       ps[:, :])
                    if STAGES == "ABCD1":
                        dbf = finp.tile([128, 512], f32, name="fin")
                        nc.vector.tensor_copy(dbf[:, :], bo_rep[:, 0:512])
                        nc.sync.dma_start(out_t[0:128, 0:512], dbf[:, :])
                        return
                    nheads = 2 if STAGES == "ABCD2" else H
                    for mc in range(4):
                        for nb2 in range(2):
                            psF = psA.tile([128, 512], f32, name="psa")
                            for h in range(nheads):
                                nc.tensor.matmul(
                                    psF[:, :],
                                    oM[h][:, mc * 128:(mc + 1) * 128],
                                    wo[h][:, nb2 * 512:(nb2 + 1) * 512],
                                    start=(h == 0), stop=(h == nheads - 1))
                            fin = finp.tile([128, 512], f32, name="fin")
                            nc.vector.tensor_add(
                                fin[:, :], psF[:, :],
                                bo_rep[:, nb2 * 512:(nb2 + 1) * 512])
                            nc.sync.dma_start(
                                out_t[mc * 128:(mc + 1) * 128,
                                      nb2 * 512:(nb2 + 1) * 512],
                                fin[:, :])

            if niter == 1:
                body()
            else:
                with tc.For_i(0, niter, 1) as iv:
                    body(iv)

    nc.finalize()
    return nc


_nc_cache = {}


def _get_nc(niter=1):
    if niter not in _nc_cache:
        _nc_cache[niter] = build_nc(niter)
    return _nc_cache[niter]


def make_in_maps(x, bias, mask, Wq, Wkv, Wo, bo):
    x = np.asarray(x, dtype=np.float32)
    bias = np.asarray(bias, dtype=np.float32)
    mask = np.asarray(mask)
    in_maps = []
    for c in range(8):
        b, qi = c // 4, c % 4
        q0 = qi * Q
        maskneg = np.where(mask[b], 0.0, MASK_NEG).astype(np.float32)
        in_maps.append({
            "x_in": np.ascontiguousarray(x[b]),
            "xq_in": np.ascontiguousarray(x[b, q0:q0 + Q]),
            "bias_in": np.ascontiguousarray(bias[b, q0:q0 + Q]),
            "maskneg_in": np.ascontiguousarray(maskneg.reshape(KC, 128).T),
            "wq_in": np.ascontiguousarray(np.asarray(Wq, dtype=np.float32)),
            "wkv_in": np.ascontiguousarray(np.asarray(Wkv, dtype=np.float32)),
            "wo_in": np.ascontiguousarray(np.asarray(Wo, dtype=np.float32)),
            "bo_in": np.ascontiguousarray(
                np.asarray(bo, dtype=np.float32).reshape(1, F)),
        })
    return in_maps


class _CachedRunner:
    """Jit the NEFF-backed executable once; repeat kernel() calls then skip
    the ~40s relower/recompile and run in ~0.1s."""

    def __init__(self, nc, n_cores=8):
        import jax
        from jax.sharding import Mesh, PartitionSpec
        from jax.experimental.shard_map import shard_map
        from concourse.bass2jax import (_bass_exec_p, install_neuronx_cc_hook,
                                        partition_id_tensor)
        install_neuronx_cc_hook()
        self.jax = jax
        self.n_cores = n_cores
        pname = nc.partition_id_tensor.name if nc.partition_id_tensor else None
        in_names, out_names, out_avals, zeros = [], [], [], []
        for alloc in nc.m.functions[0].allocations:
            if not isinstance(alloc, mybir.MemoryLocationSet):
                continue
            name = alloc.memorylocations[0].name
            if alloc.kind == "ExternalInput":
                if name != pname:
                    in_names.append(name)
            elif alloc.kind == "ExternalOutput":
                out_names.append(name)
                shape = tuple(alloc.tensor_shape)
                dt_np = mybir.dt.np(alloc.dtype)
                out_avals.append(jax.core.ShapedArray(shape, dt_np))
                zeros.append(np.zeros(shape, dt_np))
        self.in_names, self.out_names = in_names, out_names
        self.out_avals, self.zeros = out_avals, zeros
        all_names = in_names + out_names + ([pname] if pname else [])

        def _body(*args):
            ops = list(args)
            if pname is not None:
                ops.append(partition_id_tensor())
            return tuple(_bass_exec_p.bind(
                *ops, out_avals=tuple(out_avals), in_names=tuple(all_names),
                out_names=tuple(out_names), lowering_input_output_aliases=(),
                sim_require_finite=True, sim_require_nnan=True, nc=nc))

        mesh = Mesh(np.asarray(jax.devices()[:n_cores]), ("core",))
        spec_in = (PartitionSpec("core"),) * (len(in_names) + len(out_names))
        spec_out = (PartitionSpec("core"),) * len(out_names)
        self.fn = jax.jit(shard_map(_body, mesh=mesh, in_specs=spec_in,
                                    out_specs=spec_out, check_rep=False),
                          keep_unused=True)

    def run(self, in_maps):
        n = self.n_cores
        args = [np.concatenate([np.asarray(in_maps[c][k]) for c in range(n)], axis=0)
                for k in self.in_names]
        args += [np.zeros((n * z.shape[0], *z.shape[1:]), z.dtype)
                 for z in self.zeros]
        outs = self.fn(*args)
        self.jax.block_until_ready(outs)
        return [{k: np.asarray(outs[i]).reshape(n, *self.out_avals[i].shape)[c]
                 for i, k in enumerate(self.out_names)} for c in range(n)]


_runner_cache = {}


def kernel(x, bias, mask, Wq, Wkv, Wo, bo):
    in_maps = make_in_maps(x, bias, mask, Wq, Wkv, Wo, bo)
    try:
        if "r" not in _runner_cache:
            _runner_cache["r"] = _CachedRunner(_get_nc(1))
        results = _runner_cache["r"].run(in_maps)
    except Exception:
        _runner_cache.pop("r", None)
        res = run_bass_kernel_spmd(_get_nc(1), in_maps, core_ids=list(range(8)))
        results = res.results
    out = np.empty((2, NK, F), dtype=np.float32)
    for c in range(8):
        b, qi = c // 4, c % 4
        out[b, qi * Q:(qi + 1) * Q] = results[c]["out_t"]
    return out



# revision 10
# speedup vs baseline: 2.9030x; 2.9030x over previous
"""Trainium2 Bass kernel for nn_Attention_88441966559243.

Attention with additive bias [B,N,N] and per-key bool mask, fp32.
  B=2, N=2048, QD=1024, HEADS=16, DIM_HEAD=64.

Sharding: 8 cores = (batch b = core//4) x (query slice q0 = (core%4)*512).
Each core computes out[b, q0:q0+512, :] completely on-device; the host gather
is a pure concatenation. No collectives.

v2 design (vs the PE-transpose + 4x-ident-inject baseline):
  - x and the core's bias slice are uploaded pre-transposed (host layout
    change only), so no PE transposes on device.
  - EB = exp(biasT + maskneg) is precomputed once per core on ACT (exp(-30000)
    underflows to exactly 0, folding the key mask in); the softmax numerator
    is then e = exp(sim*scale) * EB with the multiply on DVE in bf16.  This
    removes the old per-tile PE identity-matmul bias injection entirely.
  - k^T slabs stay SBUF-resident: each head-pair's slab is computed on PE
    interleaved with the previous pair's attention kc-loop, so there is no
    k DRAM round-trip and the slab matmuls hide under the ACT exp stream
    (the critical path, ~1.15us per 128x1024 exp tile).
  - v' = [v | 1] in bf16 goes through DRAM once (quad-head-packed loads,
    520B rows); ACT does only exp; PSUM evacuations ride on DVE.
"""
import sys
for _p in ("/opt/trn_rl_repo", "/root/.axon_site/_ro/trn_rl_repo"):
    if _p not in sys.path:
        sys.path.insert(0, _p)

import numpy as np

import concourse.bass as bass
import concourse.mybir as mybir
from concourse import bacc
from concourse.tile import TileContext
from concourse.bass_utils import run_bass_kernel_spmd

F = 1024          # feature dim (QD == INNER)
NK = 2048         # keys (full sequence)
Q = 512           # queries per core
H = 16            # heads
D = 64            # head dim
DV = 65           # head dim + ones column
SCALE = D ** -0.5
MASK_NEG = -30000.0

FC = F // 128      # 8 feature chunks
KC = NK // 128     # 16 key chunks
HP = H // 2        # 8 head pairs
NB = NK // 512     # 4 key 512-blocks

f32 = mybir.dt.float32
fr = mybir.dt.float32r
bf16 = mybir.dt.bfloat16
AF = mybir.ActivationFunctionType

AV_LAG = 2         # av matmuls trail the exp/mul producers by this many kc


def build_nc(niter: int = 1):
    nc = bacc.Bacc(None, target_bir_lowering=False)

    xT_in = nc.dram_tensor("xT_in", [F, NK], fr, kind="ExternalInput")
    xqT_in = nc.dram_tensor("xqT_in", [F, Q], fr, kind="ExternalInput")
    biasT_in = nc.dram_tensor("biasT_in", [NK, Q], f32, kind="ExternalInput")
    maskneg_in = nc.dram_tensor("maskneg_in", [128, KC], f32, kind="ExternalInput")
    wq_in = nc.dram_tensor("wq_in", [F, F], fr, kind="ExternalInput")
    wkv_in = nc.dram_tensor("wkv_in", [F, 2 * F], fr, kind="ExternalInput")
    wo_in = nc.dram_tensor("wo_in", [F, F], fr, kind="ExternalInput")
    bo_in = nc.dram_tensor("bo_in", [1, F], fr, kind="ExternalInput")
    out_t = nc.dram_tensor("out_t", [Q, F], f32, kind="ExternalOutput")

    with TileContext(nc) as tc:
        with (
            tc.tile_pool(name="const", bufs=1) as constp,
            tc.tile_pool(name="dram", bufs=1, space="DRAM") as dramp,
            tc.tile_pool(name="ps2", bufs=2, space="PSUM") as ps2p,   # [128,1024]
            tc.tile_pool(name="psk", bufs=2, space="PSUM") as pskp,   # [128,512]
            tc.tile_pool(name="psu", bufs=2, space="PSUM") as psup,   # [DV,512]
        ):
            ones_f = constp.tile([128, 128], f32)
            nc.vector.memset(ones_f[:, :], 1.0)
            ones_r = constp.tile([128, 128], fr)
            nc.scalar.copy(ones_r[:, :], ones_f[:, :])
            masksb = constp.tile([128, KC], f32)
            nc.sync.dma_start(masksb[:, :], maskneg_in[:, :])

            outM = dramp.tile([F, Q], fr)               # merged out^T
            vprime = dramp.tile([NK, H * DV], bf16)     # v' (keys-major)

            def body(_iv=None):
                with (
                    tc.tile_pool(name="EBp", bufs=1) as EBp,
                    tc.tile_pool(name="qTp", bufs=1) as qTp,
                ):
                    EB = [EBp.tile([128, 2 * Q], bf16, tag=f"EB{i}",
                                   name=f"EB{i}") for i in range(KC)]
                    qT = [qTp.tile([128, Q], fr, tag=f"qT{i}", name=f"qT{i}")
                          for i in range(FC)]

                    with tc.tile_pool(name="xTp", bufs=1) as xTp:
                        xT = [xTp.tile([128, NK], fr, tag=f"xT{i}",
                                       name=f"xT{i}") for i in range(FC)]
                        for fc in range(FC):
                            nc.sync.dma_start(xT[fc][:, :],
                                              xT_in[fc * 128:(fc + 1) * 128, :])

                        # ---- A: q and v' projections (weights scoped) ----
                        with tc.tile_pool(name="wload", bufs=8) as wlp:
                            wq = [wlp.tile([128, F], fr, tag="w", name="w")
                                  for _ in range(FC)]
                            for fc in range(FC):
                                nc.sync.dma_start(
                                    wq[fc][:, :],
                                    wq_in[fc * 128:(fc + 1) * 128, :])
                            wv = [wlp.tile([128, F], fr, tag="w", name="w")
                                  for _ in range(FC)]
                            for fc in range(FC):
                                nc.sync.dma_start(
                                    wv[fc][:, :],
                                    wkv_in[fc * 128:(fc + 1) * 128, F:2 * F])
                            xqT = [wlp.tile([128, Q], fr, tag="xq", name="xq")
                                   for _ in range(FC)]
                            for fc in range(FC):
                                nc.sync.dma_start(
                                    xqT[fc][:, :],
                                    xqT_in[fc * 128:(fc + 1) * 128, :])

                            # A2: qT[m] = Wq[:,m]^T @ xqT  (unscaled;
                            # SCALE rides on the exp activation's scale)
                            for m in range(FC):
                                ps = pskp.tile([128, 512], f32, name="psk")
                                for fc in range(FC):
                                    nc.tensor.matmul(
                                        ps[:, :],
                                        wq[fc][:, m * 128:(m + 1) * 128],
                                        xqT[fc][:, :],
                                        start=(fc == 0), stop=(fc == FC - 1))
                                nc.vector.tensor_copy(qT[m][:, :], ps[:, :])

                            # ---- B: EB = exp(biasT + maskneg); DMAs queue
                            # behind xT/wq/wv, ACT work overlaps A4 ----
                            with tc.tile_pool(name="bT", bufs=4) as bTp:
                                for kc in range(KC):
                                    bT = bTp.tile([128, Q], f32, name="bT")
                                    nc.sync.dma_start(
                                        bT[:, :],
                                        biasT_in[kc * 128:(kc + 1) * 128, :])
                                    for half in range(2):
                                        nc.scalar.activation(
                                            EB[kc][:, half * Q:(half + 1) * Q],
                                            bT[:, :], AF.Exp,
                                            bias=masksb[:, kc:kc + 1],
                                            scale=1.0)

                            # A4: v' = [x @ Wv | 1] -> DRAM bf16, keys-major
                            with tc.tile_pool(name="vstg", bufs=3) as vstgp:
                                for kc in range(KC):
                                    vst = vstgp.tile([128, H * DV], bf16,
                                                     name="vst")
                                    nc.vector.memset(
                                        vst[:, :].rearrange(
                                            "p (h x) -> p h x",
                                            x=DV)[:, :, D:DV], 1.0)
                                    ps = ps2p.tile([128, 1024], f32,
                                                   name="ps2")
                                    for half in range(2):
                                        for fc in range(FC):
                                            nc.tensor.matmul(
                                                ps[:, half * 512:
                                                   (half + 1) * 512],
                                                xT[fc][:, kc * 128:
                                                       (kc + 1) * 128],
                                                wv[fc][:, half * 512:
                                                       (half + 1) * 512],
                                                start=(fc == 0),
                                                stop=(fc == FC - 1))
                                    nc.vector.tensor_copy(
                                        vst[:, :].rearrange(
                                            "p (h x) -> p h x",
                                            x=DV)[:, :, 0:D],
                                        ps[:, :].rearrange(
                                            "p (h d) -> p h d", d=D))
                                    nc.sync.dma_start(
                                        vprime[kc * 128:(kc + 1) * 128, :],
                                        vst[:, :])

                        # ---- C: attention; k-slab hp+1 on PE under the
                        # hp kc-loop; v' quad loads (520B rows) ----
                        with (
                            tc.tile_pool(name="wkhp", bufs=2) as wkhpp,
                            tc.tile_pool(name="kst", bufs=2) as kstp,
                            tc.tile_pool(name="vph", bufs=2) as vphp,
                            tc.tile_pool(name="eraw", bufs=3) as erawp,
                            tc.tile_pool(name="et", bufs=5) as ep,
                            tc.tile_pool(name="dsb", bufs=2) as dsbp,
                            tc.tile_pool(name="rrep", bufs=2) as rrepp,
                            tc.tile_pool(name="otst", bufs=2) as otstp,
                        ):
                            def load_wk(hp):
                                wkhp = wkhpp.tile([128, F], fr, name="wkhp")
                                nc.sync.dma_start(
                                    wkhp[:, :].rearrange(
                                        "p (fc c) -> p fc c", c=128),
                                    wkv_in[0:F, hp * 128:(hp + 1) * 128]
                                    .rearrange("(fc p) c -> p fc c", p=128))
                                return wkhp

                            def load_vquad(hq):
                                vph = vphp.tile([128, KC * 4 * DV], bf16,
                                                name="vph")
                                nc.sync.dma_start(
                                    vph[:, :].rearrange(
                                        "p (kc d) -> p kc d", d=4 * DV),
                                    vprime[:, 4 * hq * DV:(4 * hq + 4) * DV]
                                    .rearrange("(kc p) d -> p kc d", p=128))
                                return vph

                            def a3_slab(wkhp):
                                """k-slab matmuls as closures, 1 per call."""
                                kst = kstp.tile([128, NK], fr, name="kst")
                                state = {}

                                def mk(nb, fc):
                                    def run():
                                        if fc == 0:
                                            state[nb] = pskp.tile(
                                                [128, 512], f32, name="psk")
                                        nc.tensor.matmul(
                                            state[nb][:, :],
                                            wkhp[:, fc * 128:(fc + 1) * 128],
                                            xT[fc][:, nb * 512:(nb + 1) * 512],
                                            start=(fc == 0),
                                            stop=(fc == FC - 1))
                                        if fc == FC - 1:
                                            nc.vector.tensor_copy(
                                                kst[:, nb * 512:
                                                    (nb + 1) * 512],
                                                state.pop(nb)[:, :])
                                    return run

                                return kst, [mk(nb, fc) for nb in range(NB)
                                             for fc in range(FC)]

                            wk0 = load_wk(0)
                            kst_cur, ops0 = a3_slab(wk0)
                            for op in ops0:
                                op()
                            wk_next = load_wk(1)
                            vph_cur = load_vquad(0)

                            for hp in range(HP):
                                if hp % 2 == 1 and hp + 1 < HP:
                                    vph_next = load_vquad((hp + 1) // 2)
                                if hp + 1 < HP:
                                    kst_next, a3_ops = a3_slab(wk_next)
                                    if hp + 2 < HP:
                                        wk_next = load_wk(hp + 2)
                                else:
                                    kst_next, a3_ops = None, []
                                psU = [psup.tile([DV, 512], f32, name="psu")
                                       for _ in range(2)]
                                pending = []

                                def drain_av(upto):
                                    while pending and pending[0][0] <= upto:
                                        kc0, eT = pending.pop(0)
                                        for sub in range(2):
                                            hq_off = (2 * hp + sub) % 4
                                            nc.tensor.matmul(
                                                psU[sub][:, :],
                                                vph_cur[:,
                                                        kc0 * 4 * DV
                                                        + hq_off * DV:
                                                        kc0 * 4 * DV
                                                        + (hq_off + 1) * DV],
                                                eT[:, sub * Q:(sub + 1) * Q],
                                                start=(kc0 == 0),
                                                stop=(kc0 == KC - 1))

                                for kc in range(KC):
                                    ps = ps2p.tile([128, 1024], f32,
                                                   name="ps2")
                                    for sub in range(2):
                                        po = sub * 64
                                        nc.tensor.matmul(
                                            ps[:, sub * Q:(sub + 1) * Q],
                                            kst_cur[po:po + 64,
                                                    kc * 128:(kc + 1) * 128],
                                            qT[hp][po:po + 64, :],
                                            start=True, stop=True)
                                    # 2 slab matmuls for hp+1 per kc chunk
                                    for _ in range(2):
                                        if a3_ops:
                                            a3_ops.pop(0)()
                                    eRaw = erawp.tile([128, 1024], bf16,
                                                      name="eRaw")
                                    nc.scalar.activation(
                                        eRaw[:, :], ps[:, :], AF.Exp,
                                        scale=SCALE)
                                    eT = ep.tile([128, 1024], bf16,
                                                 name="eT")
                                    nc.vector.tensor_mul(eT[:, :],
                                                         eRaw[:, :],
                                                         EB[kc][:, :])
                                    pending.append((kc, eT))
                                    drain_av(kc - AV_LAG)
                                for op in a3_ops:
                                    op()
                                drain_av(KC)

                                for sub in range(2):
                                    h = 2 * hp + sub
                                    Dsb = dsbp.tile([DV, 512], fr,
                                                    name="Dsb")
                                    nc.vector.tensor_copy(
                                        Dsb[64:65, :], psU[sub][64:65, :])
                                    psR = pskp.tile([128, 512], f32,
                                                    name="psk")
                                    nc.tensor.matmul(psR[0:64, :],
                                                     ones_r[64:65, 0:64],
                                                     Dsb[64:65, :],
                                                     start=True, stop=True)
                                    rrep = rrepp.tile([64, 512], f32,
                                                      name="rrep")
                                    nc.vector.reciprocal_approx_fast(
                                        out=rrep[:, :], in_=psR[0:64, :])
                                    ot = otstp.tile([64, Q], fr, name="ot")
                                    nc.vector.tensor_mul(ot[:, :],
                                                         psU[sub][0:64, :],
                                                         rrep[:, :])
                                    nc.sync.dma_start(
                                        outM[h * 64:(h + 1) * 64, :],
                                        ot[:, :])
                                kst_cur = kst_next
                                if hp % 2 == 1 and hp + 1 < HP:
                                    vph_cur = vph_next

                # ======== stage D ========
                with (
                    tc.tile_pool(name="wop", bufs=1) as wop,
                    tc.tile_pool(name="oMp", bufs=1) as oMp,
                    tc.tile_pool(name="fin", bufs=3) as finp,
                    tc.tile_pool(name="bop", bufs=1) as bop,
                ):
                    bo_sb = bop.tile([1, F], fr, name="bo_sb")
                    nc.sync.dma_start(bo_sb[:, :], bo_in[:, :])
                    bo_rep = bop.tile([128, F], f32, name="bo_rep")
                    # head-PAIR tiles: stacking heads 2i/2i+1 on partitions
                    # 0:64/64:128 turns D into K=128 full-rate matmuls
                    wo = [wop.tile([128, F], fr, tag=f"wo{i}", name=f"wo{i}")
                          for i in range(H // 2)]
                    oM = [oMp.tile([128, Q], fr, tag=f"oM{i}", name=f"oM{i}")
                          for i in range(H // 2)]
                    for i in range(H // 2):
                        nc.sync.dma_start(wo[i][:, :],
                                          wo_in[i * 128:(i + 1) * 128, :])
                        nc.sync.dma_start(oM[i][:, :],
                                          outM[i * 128:(i + 1) * 128, :])
                    for nb2 in range(2):
                        ps = pskp.tile([128, 512], f32, name="psk")
                        nc.tensor.matmul(ps[:, :], ones_r[0:1, 0:128],
                                         bo_sb[0:1, nb2 * 512:(nb2 + 1) * 512],
                                         start=True, stop=True)
                        nc.vector.tensor_copy(
                            bo_rep[:, nb2 * 512:(nb2 + 1) * 512], ps[:, :])
                    for mc in range(4):
                        for nb2 in range(2):
                            psF = pskp.tile([128, 512], f32, name="psk")
                            for h in range(H // 2):
                                nc.tensor.matmul(
                                    psF[:, :],
                                    oM[h][:, mc * 128:(mc + 1) * 128],
                                    wo[h][:, nb2 * 512:(nb2 + 1) * 512],
                                    start=(h == 0), stop=(h == H // 2 - 1))
                            fin = finp.tile([128, 512], f32, name="fin")
                            nc.vector.tensor_add(
                                fin[:, :], psF[:, :],
                                bo_rep[:, nb2 * 512:(nb2 + 1) * 512])
                            nc.sync.dma_start(
                                out_t[mc * 128:(mc + 1) * 128,
                                      nb2 * 512:(nb2 + 1) * 512],
                                fin[:, :])

            if niter == 1:
                body()
            else:
                with tc.For_i(0, niter, 1) as iv:
                    body(iv)

    nc.finalize()
    return nc


_nc_cache = {}


def _get_nc(niter=1):
    if niter not in _nc_cache:
        _nc_cache[niter] = build_nc(niter)
    return _nc_cache[niter]


def make_in_maps(x, bias, mask, Wq, Wkv, Wo, bo):
    x = np.asarray(x, dtype=np.float32)
    bias = np.asarray(bias, dtype=np.float32)
    mask = np.asarray(mask)
    in_maps = []
    for c in range(8):
        b, qi = c // 4, c % 4
        q0 = qi * Q
        maskneg = np.where(mask[b], 0.0, MASK_NEG).astype(np.float32)
        in_maps.append({
            "xT_in": np.ascontiguousarray(x[b].T),
            "xqT_in": np.ascontiguousarray(x[b, q0:q0 + Q].T),
            "biasT_in": np.ascontiguousarray(bias[b, q0:q0 + Q].T),
            "maskneg_in": np.ascontiguousarray(maskneg.reshape(KC, 128).T),
            "wq_in": np.ascontiguousarray(np.asarray(Wq, dtype=np.float32)),
            "wkv_in": np.ascontiguousarray(np.asarray(Wkv, dtype=np.float32)),
            "wo_in": np.ascontiguousarray(np.asarray(Wo, dtype=np.float32)),
            "bo_in": np.ascontiguousarray(
                np.asarray(bo, dtype=np.float32).reshape(1, F)),
        })
    return in_maps


class _CachedRunner:
    """Jit the NEFF-backed executable once; repeat kernel() calls then skip
    the ~40s relower/recompile and run in ~0.1s."""

    def __init__(self, nc, n_cores=8):
        import jax
        from jax.sharding import Mesh, PartitionSpec
        from jax.experimental.shard_map import shard_map
        from concourse.bass2jax import (_bass_exec_p, install_neuronx_cc_hook,
                                        partition_id_tensor)
        install_neuronx_cc_hook()
        self.jax = jax
        self.n_cores = n_cores
        pname = nc.partition_id_tensor.name if nc.partition_id_tensor else None
        in_names, out_names, out_avals, zeros = [], [], [], []
        for alloc in nc.m.functions[0].allocations:
            if not isinstance(alloc, mybir.MemoryLocationSet):
                continue
            name = alloc.memorylocations[0].name
            if alloc.kind == "ExternalInput":
                if name != pname:
                    in_names.append(name)
            elif alloc.kind == "ExternalOutput":
                out_names.append(name)
                shape = tuple(alloc.tensor_shape)
                dt_np = mybir.dt.np(alloc.dtype)
                out_avals.append(jax.core.ShapedArray(shape, dt_np))
                zeros.append(np.zeros(shape, dt_np))
        self.in_names, self.out_names = in_names, out_names
        self.out_avals, self.zeros = out_avals, zeros
        all_names = in_names + out_names + ([pname] if pname else [])

        def _body(*args):
            ops = list(args)
            if pname is not None:
                ops.append(partition_id_tensor())
            return tuple(_bass_exec_p.bind(
                *ops, out_avals=tuple(out_avals), in_names=tuple(all_names),
                out_names=tuple(out_names), lowering_input_output_aliases=(),
                sim_require_finite=True, sim_require_nnan=True, nc=nc))

        mesh = Mesh(np.asarray(jax.devices()[:n_cores]), ("core",))
        spec_in = (PartitionSpec("core"),) * (len(in_names) + len(out_names))
        spec_out = (PartitionSpec("core"),) * len(out_names)
        self.fn = jax.jit(shard_map(_body, mesh=mesh, in_specs=spec_in,
                                    out_specs=spec_out, check_rep=False),
                          keep_unused=True)

    def run(self, in_maps):
        n = self.n_cores
        args = [np.concatenate([np.asarray(in_maps[c][k]) for c in range(n)], axis=0)
                for k in self.in_names]
        args += [np.zeros((n * z.shape[0], *z.shape[1:]), z.dtype)
                 for z in self.zeros]
        outs = self.fn(*args)
        self.jax.block_until_ready(outs)
        return [{k: np.asarray(outs[i]).reshape(n, *self.out_avals[i].shape)[c]
                 for i, k in enumerate(self.out_names)} for c in range(n)]


_runner_cache = {}


def kernel(x, bias, mask, Wq, Wkv, Wo, bo):
    in_maps = make_in_maps(x, bias, mask, Wq, Wkv, Wo, bo)
    try:
        if "r" not in _runner_cache:
            _runner_cache["r"] = _CachedRunner(_get_nc(1))
        results = _runner_cache["r"].run(in_maps)
    except Exception:
        _runner_cache.pop("r", None)
        res = run_bass_kernel_spmd(_get_nc(1), in_maps, core_ids=list(range(8)))
        results = res.results
    out = np.empty((2, NK, F), dtype=np.float32)
    for c in range(8):
        b, qi = c // 4, c % 4
        out[b, qi * Q:(qi + 1) * Q] = results[c]["out_t"]
    return out
